# revision 1
# baseline (speedup 1.0000x reference)
"""Multi-head attention (nn_Attention_987842478290) on 8 TRN2 NeuronCores.

Sharding: batch (2) x head-group (4 groups of 4 heads) = 8 cores; the
host pre-transposes q/k/v per batch and slices Wq/Wk/Wv columns (and Wo
rows) per head group, so every core runs the identical SPMD program on
its shard. Per core:
  - q/k/v projections on PE (bf16, fp32 PSUM); qh/kh kept TRANSPOSED
    [head-cols, tokens]; emitted as kc-half "filler quanta".
  - scores per head pair: S^T tile = [m-tile, headA-n512 | headB-n512],
    two K=64 matmuls; one 1024-wide ACT exp (scale=1/8 folded in).
  - AV with the exp tile STATIONARY: out[n-chunk, c] = sum_m pt[m,n]
    vh[m,c] accumulated over 16 m-tiles in PSUM; output free size is 65
    rows per matmul (vs 512 vh-stationary), and a ones column in the rhs
    accumulates the softmax row-sums alongside.
  - normalization: row-sums share the partition with their row -> [P,1]
    reciprocal + per-partition tensor_scalar multiply; the [n,c] tile is
    transposed back to [c,n] on the DMA XBAR (dma_start_transpose).
  - out-projection per 128-token chunk, staged bf16 and DMA'd out.
Host: sums the 4 partial outputs per batch and adds bo + bv @ Wo.

Schedule: software pipeline with phases p=0..8; phase p produces
scores+exp for group GS[p] while consuming (AV) group GS[p-1].
Projections and out-projections are emitted as ~0.5-1.7us ring-tile
quanta placed by an EDF scheduler between the per-step AV and S blocks,
keeping PE dense while ACT runs the exp stream (the model's wall floor
is ~133us of exp on ACT vs ~140us of matmul rows on PE).  A 48-matmul
N=1 warmup absorbs the post-DMA-stall dispatch burst that the cost
model prices at the LOW p-state; the last group's AV runs two steps
behind its exp stream so the epilogue is only the final drain chain.

Numerics: matmul inputs bf16 or float32r; PSUM fp32; P = exp(S) in
bf16; partial outputs bf16, summed in fp32 on the host.  Measured vs
the fp32 reference: rel err ~3.6e-3.
"""

import numpy as np
import ml_dtypes

import concourse.bass as bass
import concourse.mybir as mybir
import concourse.tile as tile
from concourse.bass_utils import run_bass_kernel_spmd
from concourse.vector_clock import ScopedClock

F32 = mybir.dt.float32
F32R = mybir.dt.float32r
BF16 = mybir.dt.bfloat16
AF = mybir.ActivationFunctionType

B, T, E = 2, 2048, 1024
HEADS, HD = 16, 64
NC_ = 8
GROUPS = 4                  # head-groups (4 heads each)
GC = 256                    # cols per core = 4 heads * 64
P = 128
KC = E // P                 # 8 contraction chunks for projections
NJ = T // 512               # 4 n-chunks of 512
SCALE = 1.0 / np.sqrt(HD)   # 1/8
GS = [(j, hp) for j in range(NJ) for hp in range(2)]  # group sequence


class SplitDrainTileContext(tile.TileContext):
    """TileContext whose final drain never carries >1 sem wait.

    This walrus build rejects >1 sync-wait per instruction; the stock
    epilogue funnels every outstanding wait onto one SP Drain. Emit the
    extra waits on individual SP nops instead.
    """

    def _drain_and_barrier(self, tick_clock, wait_clock):
        drain_inst = self.nc.sync.drain()
        wait_clock.add_sem_waits(
            drain_inst.ins, ScopedClock({None: tick_clock.global_clock})
        )
        si = drain_inst.ins.sync_info
        waits = list(si.on_wait) if si is not None else []
        if len(waits) > 1:
            import bass_rust

            si.on_wait = waits[:1]
            for w in waits[1:]:
                nop = self.nc.sync.nop(nofuse=True)
                nop.ins.sync_info = bass_rust.SyncInfo(on_wait=[w], on_update=[])

        self.nc.all_engine_barrier()
        assert self.sems is not None
        popped = self.nc._tile_sem_poison_stack.pop()
        assert popped is self._sem_poison
        self.nc.clear_and_free_semaphores(list(self.sems.allocated().values()))
        self.nc.all_engine_barrier()


def _split_multi_waits(nc):
    """Move excess sem waits onto preceding same-engine nops.

    This walrus build accepts at most one sync wait per instruction (two
    for EventSemaphore); Tile's scheduler sometimes attaches more (final
    drain, DMA WAR chains). Each engine executes its block instructions
    in list order, so a nop carrying the extra wait immediately before
    the instruction preserves semantics.
    """
    import bass_rust

    for f in nc.m.functions:
        for bb in f.blocks:
            insts = list(bb.instructions)
            out, changed = [], False
            for inst in insts:
                si = inst.sync_info
                waits = list(si.on_wait) if si is not None else []
                cap = 2 if isinstance(inst, mybir.InstEventSemaphore) else 1
                if len(waits) > cap:
                    changed = True
                    for w in waits[: len(waits) - cap]:
                        nop = mybir.InstNoOp(
                            name=f"I-splitw-{nc.next_id()}",
                            ins=[],
                            outs=[],
                        )
                        nop.engine = inst.engine
                        nop.sync_info = bass_rust.SyncInfo(
                            on_wait=[w], on_update=[]
                        )
                        nc.register_instruction(nop, overwrite=True)
                        out.append(nop)
                    si.on_wait = waits[len(waits) - cap :]
                out.append(inst)
            if changed:
                bb.instructions = out


def build_nc() -> bass.Bass:
    nc = bass.Bass("TRN2", target_bir_lowering=False, debug=False)

    qT = nc.dram_tensor("qT", [E, T], BF16, kind="ExternalInput").ap()
    kT = nc.dram_tensor("kT", [E, T], BF16, kind="ExternalInput").ap()
    vT = nc.dram_tensor("vT", [E, T], BF16, kind="ExternalInput").ap()
    wq = nc.dram_tensor("wq", [E, GC], BF16, kind="ExternalInput").ap()
    wk = nc.dram_tensor("wk", [E, GC], BF16, kind="ExternalInput").ap()
    wv = nc.dram_tensor("wv", [E, GC], BF16, kind="ExternalInput").ap()
    wo = nc.dram_tensor("wo", [GC, E], BF16, kind="ExternalInput").ap()
    bq = nc.dram_tensor("bq", [GC], F32, kind="ExternalInput").ap()
    bk = nc.dram_tensor("bk", [GC], F32, kind="ExternalInput").ap()
    out = nc.dram_tensor("out", [T, E], BF16, kind="ExternalOutput").ap()

    with SplitDrainTileContext(nc) as tc:
        _build_body(nc, tc, qT, kT, vT, wq, wk, wv, wo, bq, bk, out)
    _split_multi_waits(nc)
    return nc


def _build_body(nc, tc, qT, kT, vT, wq, wk, wv, wo, bq, bk, out):
    from contextlib import ExitStack

    ctx = ExitStack()
    with ctx:
        cpool = ctx.enter_context(tc.tile_pool(name="consts", bufs=1))
        xpool = ctx.enter_context(tc.tile_pool(name="xstream", bufs=5))
        vpool = ctx.enter_context(tc.tile_pool(name="vstream", bufs=4))
        ptpool = ctx.enter_context(tc.tile_pool(name="pt", bufs=24))
        anpool = ctx.enter_context(tc.tile_pool(name="an", bufs=12))
        rpool = ctx.enter_context(tc.tile_pool(name="rec", bufs=6))
        opool = ctx.enter_context(tc.tile_pool(name="ostage", bufs=3))
        # PSUM: shared ring (scores / projections / out-proj) 2x2 banks,
        # AV accumulators 2x2 banks = all 8 banks.
        psR = ctx.enter_context(tc.tile_pool(name="psR", bufs=2, space="PSUM"))
        psA = ctx.enter_context(tc.tile_pool(name="psA", bufs=2, space="PSUM"))

        # ---- persistent tiles ----
        wk_sb = cpool.tile([P, KC, GC], BF16, tag="wk")
        wq_sb = cpool.tile([P, KC, GC], BF16, tag="wq")
        wv_sb = cpool.tile([P, KC, GC], BF16, tag="wv")
        bq_sb = cpool.tile([P, 2], F32, tag="bq")
        bk_sb = cpool.tile([P, 2], F32, tag="bk")
        wo_sb = cpool.tile([P, 2, E], BF16, tag="wo")

        qhB = [
            [cpool.tile([P, 512], F32R, tag=f"qh{hp}_{j}", name=f"qh{hp}_{j}") for j in range(NJ)]
            for hp in range(2)
        ]
        khB = [
            [cpool.tile([P, 512], F32R, tag=f"kh{hp}_{j}", name=f"kh{hp}_{j}") for j in range(NJ)]
            for hp in range(2)
        ]
        vh1 = [
            cpool.tile([P, 4, HD + 1], BF16, tag=f"vh1_{i}", name=f"vh1_{i}") for i in range(16)
        ]
        # attT[hp][cb]: [head-cols 128, tokens 128] for token chunk cb
        attT = [
            [cpool.tile([P, P], BF16, tag=f"attT{hp}_{cb}", name=f"attT{hp}_{cb}") for cb in range(16)]
            for hp in range(2)
        ]

        for i in range(16):
            nc.vector.memset(vh1[i][:, :, HD : HD + 1], 1.0)

        xk, xq, xv = {}, {}, {}

        def load_half(dram, j, kh, pool, tag):
            """kc-half load: E-rows kh*512..+512, token block j (1KB elems)."""
            t = pool.tile([P, 4, 512], BF16, tag=f"{tag}{kh}", name=f"{tag}{j}{kh}")
            nc.sync.dma_start(
                t[:],
                dram[
                    kh * 512 : (kh + 1) * 512, j * 512 : (j + 1) * 512
                ].rearrange("(kc p) t -> p kc t", p=P),
            )
            return t

        # DMA emission order = SP dispatch order (due-ordered).
        nc.sync.dma_start(wk_sb[:], wk.rearrange("(kc p) c -> p kc c", p=P))
        nc.sync.dma_start(bk_sb[:], bk.rearrange("(hp p) -> p hp", p=P))
        xk[0] = [load_half(kT, 0, kh, xpool, "xb") for kh in range(2)]
        nc.sync.dma_start(wq_sb[:], wq.rearrange("(kc p) c -> p kc c", p=P))
        nc.sync.dma_start(bq_sb[:], bq.rearrange("(hp p) -> p hp", p=P))
        xq[0] = [load_half(qT, 0, kh, xpool, "xb") for kh in range(2)]
        xk[1] = [load_half(kT, 1, kh, xpool, "xb") for kh in range(2)]
        nc.sync.dma_start(wv_sb[:], wv.rearrange("(kc p) c -> p kc c", p=P))
        xk[2] = [load_half(kT, 2, kh, xpool, "xb") for kh in range(2)]
        xv[0] = [load_half(vT, 0, kh, vpool, "vb") for kh in range(2)]
        xk[3] = [load_half(kT, 3, kh, xpool, "xb") for kh in range(2)]
        xv[1] = [load_half(vT, 1, kh, vpool, "vb") for kh in range(2)]
        xq[1] = [load_half(qT, 1, kh, xpool, "xb") for kh in range(2)]
        xv[2] = [load_half(vT, 2, kh, vpool, "vb") for kh in range(2)]
        xv[3] = [load_half(vT, 3, kh, vpool, "vb") for kh in range(2)]
        for j in (2, 3):
            xq[j] = [load_half(qT, j, kh, xpool, "xb") for kh in range(2)]
        nc.sync.dma_start(wo_sb[:], wo.rearrange("(kk p) e -> p kk e", p=P))

        # ---- building blocks ----
        pts = {}

        def s_step(g, i):
            """Scores + exp for m-tile i of group g (PE 2 x [128,512], ACT)."""
            j, hp = g
            S = psR.tile([P, 1024], F32, tag="ring", name=f"S{j}{hp}_{i}")
            for hb in range(2):
                cs = slice(hb * HD, (hb + 1) * HD)
                nc.tensor.matmul(
                    S[:, hb * 512 : (hb + 1) * 512],
                    lhsT=khB[hp][i // 4][cs, (i % 4) * P : (i % 4 + 1) * P],
                    rhs=qhB[hp][j][cs, :],
                    start=True,
                    stop=True,
                )
            pt = ptpool.tile([P, 1024], BF16, tag="pt", name=f"pt{j}{hp}_{i}")
            nc.scalar.activation(pt[:], S[:], AF.Exp, scale=SCALE)
            pts[(j, hp, i)] = pt

        def av_mm(g, i, acc, ch, hb):
            j, hp = g
            r = 2 * ch + hb
            # ZERO_REGION granularity is a full 2KB bank: only the first
            # region per bank carries start=True; the bank-wide pending-zero
            # covers the siblings.
            nc.tensor.matmul(
                acc[:, r // 4, r % 4, 0 : HD + 1],
                lhsT=pts[(j, hp, i)][
                    :, hb * 512 + ch * P : hb * 512 + (ch + 1) * P
                ],
                rhs=vh1[i][:, 2 * hp + hb, :],
                start=(i == 0 and r % 4 == 0),
                stop=(i == 15),
            )

        def av_step(g, i, acc):
            """AV for m-tile i: 8 matmuls [n128, 65], pt stationary."""
            for ch in range(4):
                for hb in range(2):
                    av_mm(g, i, acc, ch, hb)
            pts.pop(g + (i,))

        def drain_chunk(g, acc, ch):
            """Normalize (DVE) + XBAR-transpose (DMA) one 128-token chunk."""
            j, hp = g
            b, k0 = (2 * ch) // 4, (2 * ch) % 4
            rec = rpool.tile(
                [P, 1, 2, 1], F32, tag="rec", name=f"rec{j}{hp}{ch}"
            )
            nc.vector.reciprocal(
                rec[:], acc[:, b : b + 1, k0 : k0 + 2, HD : HD + 1]
            )
            an = anpool.tile([P, P], BF16, tag="an", name=f"an{j}{hp}{ch}")
            for hb in range(2):
                nc.vector.tensor_scalar_mul(
                    an[:, hb * HD : (hb + 1) * HD],
                    acc[:, b, k0 + hb, 0:HD],
                    rec[:, 0, hb, :],
                )
            nc.sync.dma_start_transpose(attT[hp][4 * j + ch][:], an[:])

        def drain(g, acc):
            for ch in range(4):
                drain_chunk(g, acc, ch)

        def outproj_quantum(j, ch, e2, po):
            cb = 4 * j + ch
            for kk in range(2):
                nc.tensor.matmul(
                    po[:, e2 * 512 : (e2 + 1) * 512],
                    lhsT=attT[kk][cb][:],
                    rhs=wo_sb[:, kk, e2 * 512 : (e2 + 1) * 512],
                    start=(kk == 0),
                    stop=(kk == 1),
                )

        def outproj_drain(j, ch, po):
            cb = 4 * j + ch
            ost = opool.tile([P, E], BF16, tag="ost", name=f"ost{cb}")
            nc.vector.tensor_copy(out=ost[:], in_=po[:])
            nc.sync.dma_start(out[cb * P : (cb + 1) * P, :], ost[:])

        # ---- filler tiles: PE quanta, 2 per ring tile ----
        def mk_qkproj(xhalves, w_sb, b_sb, dst, j, hp):
            """Projection tile for (tensor, j, hp): two kc-half quanta."""
            state = {}

            def quantum(kh):
                def fn():
                    if "ps" not in state:
                        state["ps"] = psR.tile(
                            [P, 1024], F32, tag="ring", name=f"pj{j}{hp}{kh}"
                        )
                    ps = state["ps"]
                    for kc in range(4):
                        nc.tensor.matmul(
                            ps[:, 0:512],
                            lhsT=w_sb[:, 4 * kh + kc, hp * P : (hp + 1) * P],
                            rhs=xhalves[kh][:, kc, :],
                            start=(kh == 0 and kc == 0),
                            stop=(kh == 1 and kc == 3),
                        )
                    if kh == 1:
                        nc.vector.tensor_scalar_add(
                            dst[hp][j][:], ps[:, 0:512], b_sb[:, hp : hp + 1]
                        )
                return fn

            return [quantum(0), quantum(1)]

        def mk_vproj(ib, ii):
            """v-projection m-tile 4*ib+ii as two kc-half quanta."""
            i = 4 * ib + ii
            state = {}

            def quantum(kh):
                def fn():
                    if "ps" not in state:
                        state["ps"] = psR.tile(
                            [P, 1024], F32, tag="ring", name=f"pv{i}"
                        )
                    ps = state["ps"]
                    for kc in range(4):
                        nc.tensor.matmul(
                            ps[:, 0:GC],
                            lhsT=xv[ib][kh][:, kc, ii * P : (ii + 1) * P],
                            rhs=wv_sb[:, 4 * kh + kc, :],
                            start=(kh == 0 and kc == 0),
                            stop=(kh == 1 and kc == 3),
                        )
                    if kh == 1:
                        nc.vector.tensor_copy(
                            out=vh1[i][:, :, 0:HD],
                            in_=ps[:, 0:GC].rearrange("p (h c) -> p h c", h=4),
                        )
                return fn

            return [quantum(0), quantum(1)]

        def mk_outproj(j, ch):
            state = {}

            def quantum(e2):
                def fn():
                    if "ps" not in state:
                        state["ps"] = psR.tile(
                            [P, 1024], F32, tag="ring", name=f"po{4*j+ch}"
                        )
                    outproj_quantum(j, ch, e2, state["ps"])
                    if e2 == 1:
                        outproj_drain(j, ch, state["ps"])
                return fn

            return [quantum(0), quantum(1)]

        # ---- prologue P0: block-0 hp=0 projections ----
        # Warmup: the first PE dispatch burst after the k0a DMA stall is
        # priced at the LOW p-state; burn it on N=1 matmuls (1.5ns each)
        # so the real projection matmuls dispatch at MID/FULL rate.
        warm_ps = psR.tile([P, 1024], F32, tag="ring", name="warm")
        for w in range(40):
            nc.tensor.matmul(
                warm_ps[0:8, 512:513],
                lhsT=xk[0][0][:, 0, 0:8],
                rhs=xk[0][0][:, 0, 0:1],
                start=(w == 0),
                stop=(w == 29),
            )
        for fn in mk_qkproj(xk[0], wk_sb, bk_sb, khB, 0, 0):
            fn()
        for w in range(16):
            nc.tensor.matmul(
                warm_ps[0:8, 513:514],
                lhsT=xq[0][0][:, 0, 0:8],
                rhs=xq[0][0][:, 0, 0:1],
                start=(w == 0),
                stop=(w == 15),
            )
        for fn in mk_qkproj(xq[0], wq_sb, bq_sb, qhB, 0, 0):
            fn()

        # ---- filler schedule (due/avail in global steps s = 16*phase+i) ----
        tiles = []

        def add_tile(due, avail, quanta):
            tiles.append({"due": due, "avail": avail, "quanta": quanta})

        for jb in (1, 2, 3):  # k-proj hp=0 (g0's score stream)
            add_tile(4 * jb - 1, max(0, 4 * jb - 6),
                     mk_qkproj(xk[jb], wk_sb, bk_sb, khB, jb, 0))
        add_tile(12, 8, mk_qkproj(xk[0], wk_sb, bk_sb, khB, 0, 1))
        add_tile(12, 8, mk_qkproj(xq[0], wq_sb, bq_sb, qhB, 0, 1))
        for jb in (1, 2, 3):  # k-proj hp=1 (g1's stream, one phase later)
            add_tile(16 + 4 * jb - 3, 8 + 4 * jb,
                     mk_qkproj(xk[jb], wk_sb, bk_sb, khB, jb, 1))
        for ib in range(4):
            for ii in range(4):
                add_tile(15 + 4 * ib + ii, max(0, 5 + 4 * ib + ii),
                         mk_vproj(ib, ii))
        for jq in (1, 2, 3):
            for hp in range(2):
                add_tile(16 * (2 * jq + hp) - 5, 16 * jq + 8 * hp,
                         mk_qkproj(xq[jq], wq_sb, bq_sb, qhB, jq, hp))
        for j in range(3):
            for ch in range(4):
                add_tile(None, 16 * (2 * j + 3) + ch, mk_outproj(j, ch))

        tiles.sort(key=lambda t: (t["due"] is None, t["due"] or 0, t["avail"]))
        pending = list(tiles)

        state = {"open": None}

        def emit_one(s):
            t = state["open"]
            if t is None:
                for cand in pending:
                    if cand["avail"] <= s:
                        pending.remove(cand)
                        t = cand
                        state["open"] = t
                        break
                else:
                    return False
            t["quanta"].pop(0)()
            if not t["quanta"]:
                state["open"] = None
            return True

        def fillers(s):
            # EDF feasibility at one quantum/step
            need = 0
            cum = 0
            chain = ([state["open"]] if state["open"] else []) + pending
            for t in chain:
                cum += len(t["quanta"])
                if t["due"] is not None:
                    need = max(need, cum - max(0, t["due"] - s))
            n = max(1 if (s % 2 == 0 or s >= 32) else 0, need)
            for _ in range(n):
                if not emit_one(s):
                    break

        # ---- software-pipelined main loop ----
        accs = {}
        acc8 = None
        for p in range(8):
            if p >= 1:
                g = GS[p - 1]
                accs[g] = psA.tile([P, 2, 4, P], F32, tag="acc", name=f"acc{p}")
            if p == 7:
                acc8 = psA.tile([P, 2, 4, P], F32, tag="acc", name="acc8")
            for i in range(16):
                s = 16 * p + i
                if p == 7:
                    s_step(GS[p], i)
                if p >= 1:
                    av_step(GS[p - 1], i, accs[GS[p - 1]])
                if p != 7:
                    s_step(GS[p], i)
                if p == 7 and i >= 1:
                    # head-start on the last group (consumed in the epilogue)
                    for ch in range(4):
                        for hb in range(2):
                            av_mm(GS[7], i - 1, acc8, ch, hb)
                fillers(s)
            if p >= 1:
                drain(GS[p - 1], accs[GS[p - 1]])

        # ---- epilogue: finish last group, pipelined drains + out-proj ----
        g7 = GS[7]
        for i in (15,):
            for ch in range(4):
                for hb in range(2):
                    av_mm(g7, i, acc8, ch, hb)
        j7, hp7 = g7
        rec = rpool.tile([P, 2, 4, 1], F32, tag="rec", name="rec_epi")
        nc.vector.reciprocal(rec[:], acc8[:, :, :, HD : HD + 1])
        for ch in range(4):
            b, k0 = (2 * ch) // 4, (2 * ch) % 4
            an = anpool.tile([P, P], BF16, tag="an", name=f"an_epi{ch}")
            for hb in range(2):
                nc.vector.tensor_scalar_mul(
                    an[:, hb * HD : (hb + 1) * HD],
                    acc8[:, b, k0 + hb, 0:HD],
                    rec[:, b, k0 + hb, :],
                )
            nc.sync.dma_start_transpose(attT[hp7][4 * j7 + ch][:], an[:])
        for ch in range(4):
            po = psR.tile([P, 1024], F32, tag="ring", name=f"po{12 + ch}")
            outproj_quantum(3, ch, 0, po)
            ost = opool.tile([P, E], BF16, tag="ost", name=f"ost{12 + ch}")
            nc.vector.tensor_copy(out=ost[:, 0:512], in_=po[:, 0:512])
            nc.sync.dma_start(
                out[(12 + ch) * P : (13 + ch) * P, 0:512], ost[:, 0:512]
            )
            outproj_quantum(3, ch, 1, po)
            nc.scalar.copy(out=ost[:, 512:1024], in_=po[:, 512:1024])
            nc.sync.dma_start(
                out[(12 + ch) * P : (13 + ch) * P, 512:1024], ost[:, 512:1024]
            )
        for i in range(16):
            pts.pop(g7 + (i,))


_NC_CACHE: list = []


def kernel(q, k, v, Wq, bq, Wk, bk, Wv, bv, Wo, bo):
    q = np.asarray(q, dtype=np.float32)
    k = np.asarray(k, dtype=np.float32)
    v = np.asarray(v, dtype=np.float32)
    Wq = np.asarray(Wq, dtype=np.float32)
    Wk = np.asarray(Wk, dtype=np.float32)
    Wv = np.asarray(Wv, dtype=np.float32)
    Wo = np.asarray(Wo, dtype=np.float32)
    bq = np.asarray(bq, dtype=np.float32)
    bk = np.asarray(bk, dtype=np.float32)
    bv = np.asarray(bv, dtype=np.float32)
    bo = np.asarray(bo, dtype=np.float32)

    if not _NC_CACHE:
        _NC_CACHE.append(build_nc())
    nc = _NC_CACHE[0]

    bf = ml_dtypes.bfloat16
    qTb = [np.ascontiguousarray(q[b].T).astype(bf) for b in range(B)]
    kTb = [np.ascontiguousarray(k[b].T).astype(bf) for b in range(B)]
    vTb = [np.ascontiguousarray(v[b].T).astype(bf) for b in range(B)]

    in_maps = []
    for c in range(NC_):
        b, g = divmod(c, GROUPS)
        cs = slice(g * GC, (g + 1) * GC)
        in_maps.append(
            {
                "qT": qTb[b],
                "kT": kTb[b],
                "vT": vTb[b],
                "wq": Wq[:, cs].astype(bf),
                "wk": Wk[:, cs].astype(bf),
                "wv": Wv[:, cs].astype(bf),
                "wo": np.ascontiguousarray(Wo[cs, :]).astype(bf),
                "bq": bq[cs],
                "bk": bk[cs],
            }
        )

    kw = {}
    if TRACE:
        kw = dict(trace=True, tmpdir=TRACE_DIR, **TRACE_KW)
    res = run_bass_kernel_spmd(nc, in_maps, core_ids=list(range(NC_)), **kw)
    LAST_RESULT.clear()
    LAST_RESULT.append(res)

    outp = np.zeros((B, T, E), dtype=np.float32)
    for c in range(NC_):
        b = c // GROUPS
        outp[b] += res.results[c]["out"].astype(np.float32)
    # bv's contribution (softmax rows sum to 1): (1 . bv^T) @ Wo, plus bo
    outp += bo + bv @ Wo
    return outp


TRACE = False
TRACE_DIR = None
TRACE_KW: dict = {}
LAST_RESULT: list = []

